# revision 1
# baseline (speedup 1.0000x reference)
"""HBiLSTM Trainium2 kernel.

Strategy (8 NeuronCores):
  - cores 0-3: forward LSTM + fwd highway half, 8 samples each
  - cores 4-7: backward LSTM on host-reversed input + bwd highway half
  All cores run the SAME SPMD program; direction is encoded purely in the
  per-core input data (weights + pre-reversed/pre-transposed x).

Device layout is "transposed" (layout T): hidden/gate dims on SBUF
partitions, batch on the free dim.  Host does all transposes / reversal /
concat / masking (untimed).

Phases on device, per core (8 samples, T=512, DIN=512, H=256):
  A: xg.T = Wp @ x.T + b  (Wp = [Wih(perm); Wg_half]  -> 10 gate tiles of 128)
  B: 512-step LSTM recurrence, Whh.T stationary (bf16, FWL), 2 interleaved
     chains of 4 samples to hide the per-step dependency-chain latency.
  C: highway gate flow = g_pre + sig(g_pre) * (y - g_pre), bulk, then DMA out.
"""

import numpy as np
import ml_dtypes

bf16 = ml_dtypes.bfloat16

B, T, DIN, H = 32, 512, 512, 256
NG = 4 * H          # 1024 gate rows per direction
NP = NG + H         # 1280 = gates + highway-half rows
BPC = 8             # samples per core
NCORES = 8
TOK = BPC * T       # tokens per core = 4096

# gate reorder: torch order i,f,g,o -> i,f,o,g  (so sigmoid gates are tiles 0:6,
# tanh gate is tiles 6:8 when viewed as 8 tiles of 128)
_PERM = np.concatenate([np.arange(0, 512), np.arange(768, 1024), np.arange(512, 768)])

_PROG_CACHE = {}


def _build_program(n_steps=T, static=True, unroll=16, nchain=2):
    import concourse.bacc as bacc
    import concourse.mybir as mybir
    import concourse.tile as tile
    import concourse.bass as bass

    fp32 = mybir.dt.float32
    b16 = mybir.dt.bfloat16

    nc = bacc.Bacc(None)

    xt_d = nc.dram_tensor("xt", [DIN, TOK], b16, kind="ExternalInput")
    wpt_d = nc.dram_tensor("wpt", [DIN, NP], b16, kind="ExternalInput")
    whht_d = nc.dram_tensor("whht", [H, NG], b16, kind="ExternalInput")
    bias_d = nc.dram_tensor("bias", [NP], fp32, kind="ExternalInput")
    out_d = nc.dram_tensor("out", [128, 2, T, BPC], fp32, kind="ExternalOutput")

    KT_A = DIN // 128      # 4 contraction tiles in phase A
    MT_A = NP // 128       # 10 output tiles in phase A (8 xg + 2 gpre)
    NCH_A = TOK // 512     # 8 token chunks of 512
    GT = NG // 128         # 8 gate tiles in recurrence
    KT_B = H // 128        # 2 contraction tiles in recurrence
    NCHAIN = nchain
    CB = BPC // NCHAIN

    with tile.TileContext(nc) as tc:
      with (
          tc.tile_pool(name="persist", bufs=1) as pp,
          tc.tile_pool(name="psum", bufs=2, space="PSUM") as psp,
      ):
        gpre = pp.tile([128, 2, T, BPC], fp32, tag="gpre")      # 32KB/p
        bias_sb = pp.tile([128, MT_A], fp32, tag="bias")
        nc.sync.dma_start(bias_sb[:], bias_d.rearrange("(m p) -> p m", p=128))

        whh_sb = pp.tile([128, KT_B, NG], b16, tag="whh")
        nc.sync.dma_start(whh_sb[:], whht_d.rearrange("(k p) m -> p k m", p=128))

        yh = [
            pp.tile([128, KT_B, n_steps + 1, CB], b16, tag=f"yh{ch}", name=f"yh{ch}")
            for ch in range(NCHAIN)
        ]
        cst = [
            pp.tile([128, KT_B, 1, CB], fp32, tag=f"c{ch}", name=f"c{ch}")
            for ch in range(NCHAIN)
        ]
        for ch in range(NCHAIN):
            nc.gpsimd.memset(yh[ch][:, :, 0, :], 0.0)
            nc.gpsimd.memset(cst[ch][:], 0.0)

        with tc.tile_pool(name="pxg", bufs=1) as pxg:
            xg = pxg.tile([128, GT, T, BPC], fp32, tag="xg")    # 128KB/p

            # ---------------- Phase A: projections ----------------
            with tc.tile_pool(name="phaseA", bufs=2) as pa:
                wp_sb = pa.tile([128, KT_A, NP], b16, tag="wp", bufs=1)
                nc.sync.dma_start(
                    wp_sb[:], wpt_d.rearrange("(k p) m -> p k m", p=128)
                )
                for n in range(NCH_A):
                    xt_sb = pa.tile([128, KT_A, 512], b16, tag="xt")
                    nc.sync.dma_start(
                        xt_sb[:],
                        xt_d.rearrange("(k p) n -> p k n", p=128)[
                            :, :, n * 512 : (n + 1) * 512
                        ],
                    )
                    for m in range(MT_A):
                        ps = psp.tile([128, 512], fp32, tag="psA", bufs=2)
                        for k in range(KT_A):
                            nc.tensor.matmul(
                                ps[:],
                                wp_sb[:, k, m * 128 : (m + 1) * 128],
                                xt_sb[:, k, :],
                                start=(k == 0),
                                stop=(k == KT_A - 1),
                            )
                        tchunk = ps[:].rearrange("p (t b) -> p t b", b=BPC)
                        t0 = n * (512 // BPC)
                        t1 = (n + 1) * (512 // BPC)
                        if m < GT:
                            dst = xg[:, m, t0:t1, :]
                        else:
                            dst = gpre[:, m - GT, t0:t1, :]
                        nc.vector.tensor_scalar_add(dst, tchunk, bias_sb[:, m : m + 1])

            # ---------------- Phase B: recurrence ----------------
            with tc.tile_pool(name="phaseB", bufs=6) as pb:

                def step(t):
                    if static:
                        tsl = lambda off: slice(t + off, t + off + 1)
                    else:
                        tsl = lambda off: bass.ds(t + off, 1)
                    for ch in range(NCHAIN):
                        cb = ch * CB
                        ps = psp.tile(
                            [128, GT, 1, CB], fp32, tag=f"psB{ch}", bufs=3, name=f"psB{ch}"
                        )
                        for m in range(GT):
                            for k in range(KT_B):
                                nc.tensor.matmul(
                                    ps[:, m, :, :],
                                    whh_sb[:, k, m * 128 : (m + 1) * 128],
                                    yh[ch][:, k, tsl(0), :],
                                    start=(k == 0),
                                    stop=(k == KT_B - 1),
                                )
                        gf = pb.tile([128, GT, 1, CB], fp32, tag=f"gf{ch}", name=f"gf{ch}")
                        nc.vector.tensor_add(
                            gf[:], ps[:], xg[:, :, tsl(0), cb : cb + CB]
                        )
                        sig = pb.tile([128, 6, 1, CB], fp32, tag=f"sig{ch}", name=f"sig{ch}")
                        nc.scalar.activation(
                            sig[:], gf[:, 0:6, :, :],
                            mybir.ActivationFunctionType.Sigmoid,
                        )
                        tgg = pb.tile([128, 2, 1, CB], fp32, tag=f"tg{ch}", name=f"tg{ch}")
                        nc.scalar.activation(
                            tgg[:], gf[:, 6:8, :, :],
                            mybir.ActivationFunctionType.Tanh,
                        )
                        t1_ = pb.tile([128, 2, 1, CB], fp32, tag=f"t1{ch}", name=f"t1{ch}")
                        nc.vector.tensor_mul(t1_[:], sig[:, 0:2, :, :], tgg[:])
                        t2_ = pb.tile([128, 2, 1, CB], fp32, tag=f"t2{ch}", name=f"t2{ch}")
                        nc.vector.tensor_mul(t2_[:], sig[:, 2:4, :, :], cst[ch][:])
                        nc.vector.tensor_add(cst[ch][:], t1_[:], t2_[:])
                        tau = pb.tile([128, 2, 1, CB], fp32, tag=f"tau{ch}", name=f"tau{ch}")
                        nc.scalar.activation(
                            tau[:], cst[ch][:], mybir.ActivationFunctionType.Tanh,
                        )
                        nc.vector.tensor_mul(
                            yh[ch][:, :, tsl(1), :],
                            sig[:, 4:6, :, :],
                            tau[:],
                        )

                if static:
                    for t in range(n_steps):
                        step(t)
                else:
                    tc.For_i_unrolled(0, n_steps, 1, step, max_unroll=unroll)

        # ---------------- Phase C: highway gate ----------------
        with tc.tile_pool(name="phaseC", bufs=2) as pc:
            TC = 128
            for cch in range(T // TC):
                t0, t1 = cch * TC, (cch + 1) * TC
                gp = gpre[:, :, t0:t1, :]
                tg = pc.tile([128, 2, TC, BPC], fp32, tag="tg_c")
                nc.scalar.activation(tg[:], gp, mybir.ActivationFunctionType.Sigmoid)
                yc = pc.tile([128, 2, TC, BPC], fp32, tag="y_c")
                for ch in range(NCHAIN):
                    cb = ch * CB
                    nc.vector.tensor_sub(
                        yc[:, :, :, cb : cb + CB],
                        yh[ch][:, :, t0 + 1 : t1 + 1, :],
                        gp[:, :, :, cb : cb + CB],
                    )
                fl = pc.tile([128, 2, TC, BPC], fp32, tag="fl_c")
                nc.vector.tensor_mul(fl[:], tg[:], yc[:])
                nc.vector.tensor_add(fl[:], fl[:], gp)
                nc.sync.dma_start(out_d[:, :, t0:t1, :], fl[:])

    nc.compile()
    return nc


def _reverse_padded_np(x, lens):
    t = np.arange(T)
    idx = np.where(t[None, :] < lens[:, None], lens[:, None] - 1 - t[None, :], t[None, :])
    return np.take_along_axis(x, idx[:, :, None], axis=1), idx


def kernel(x, Wih_f, Whh_f, bih_f, bhh_f, Wih_b, Whh_b, bih_b, bhh_b, Wg, bg,
           x_lengths, **_unused):
    from concourse.bass_utils import run_bass_kernel_spmd

    x = np.asarray(x, dtype=np.float32)
    lens = np.asarray(x_lengths).astype(np.int64)

    xr, idx = _reverse_padded_np(x, lens)

    def dir_weights(Wih, Whh, bih, bhh, wg_half, bg_half):
        Wp = np.concatenate([np.asarray(Wih)[_PERM], wg_half], axis=0)  # [1280, 512]
        wpt = np.ascontiguousarray(Wp.T).astype(bf16)                   # [512, 1280]
        whht = np.ascontiguousarray(np.asarray(Whh)[_PERM].T).astype(bf16)  # [256,1024]
        bias = np.concatenate(
            [(np.asarray(bih) + np.asarray(bhh))[_PERM], bg_half]
        ).astype(np.float32)
        return wpt, whht, bias

    Wg = np.asarray(Wg); bg = np.asarray(bg)
    fw = dir_weights(Wih_f, Whh_f, bih_f, bhh_f, Wg[0:H], bg[0:H])
    bw = dir_weights(Wih_b, Whh_b, bih_b, bhh_b, Wg[H:2*H], bg[H:2*H])

    in_maps = []
    for c in range(NCORES):
        fwd = c < 4
        s0 = (c % 4) * BPC
        xsrc = x if fwd else xr
        xt = np.ascontiguousarray(
            xsrc[s0 : s0 + BPC].transpose(2, 1, 0).reshape(DIN, TOK)
        ).astype(bf16)
        wpt, whht, bias = fw if fwd else bw
        in_maps.append({"xt": xt, "wpt": wpt, "whht": whht, "bias": bias})

    if "prog" not in _PROG_CACHE:
        _PROG_CACHE["prog"] = _build_program()
    nc = _PROG_CACHE["prog"]
    _PROG_CACHE["last_inmaps"] = in_maps

    res = run_bass_kernel_spmd(nc, in_maps, core_ids=list(range(NCORES)))

    full = np.zeros((B, T, 2 * H), dtype=np.float32)
    for c in range(NCORES):
        arr = np.asarray(res.results[c]["out"], dtype=np.float32)  # [128,2,T,BPC]
        half = arr.transpose(3, 2, 1, 0).reshape(BPC, T, H)
        s0 = (c % 4) * BPC
        if c < 4:
            full[s0 : s0 + BPC, :, 0:H] = half
        else:
            # un-reverse within valid lengths
            half = np.take_along_axis(half, idx[s0 : s0 + BPC][:, :, None], axis=1)
            full[s0 : s0 + BPC, :, H : 2 * H] = half

    mask = (np.arange(T)[None, :] < lens[:, None])[:, :, None]
    full *= mask
    return full



# revision 3
# speedup vs baseline: 1.4890x; 1.4890x over previous
"""HBiLSTM Trainium2 kernel (v2 — latency-optimized recurrence).

Strategy (8 NeuronCores):
  - cores 0-3: forward LSTM + fwd highway half, 8 samples each
  - cores 4-7: backward LSTM on host-reversed input + bwd highway half
  All cores run the SAME SPMD program; direction is encoded purely in the
  per-core input data (weights + pre-reversed/pre-transposed x).

Device layout: hidden/gate dims on SBUF partitions, batch on the free dim.
Host does all transposes / reversal / concat / masking (untimed).

v2 recurrence redesign (the 512-step serial chain dominates wall time):
  - ONE chain of 8 samples (v1's 2 interleaved chains added cross-chain
    queueing on DVE without shortening the serial path).
  - xg[t] is injected into the PSUM accumulation by an identity-weight
    matmul issued BEFORE the h-dependent matmuls (off the critical path),
    killing the per-step DVE add.
  - all-sigmoid ops are computed via tanh:  sigma(x) = (tanh(x/2)+1)/2,
    with the 1/2 pre-folded into the f,i,o rows of Wih/Whh/bias, so ONE
    tanh ACT call covers every gate.  Storing C=2c and H=2h (Whh columns
    pre-scaled by 1/2 to compensate) turns the whole cell update into 3
    fused scalar_tensor_tensor ops + 1 tanh:
        u       = (th_[f,i] + 1) * [C, th_g]     (paired, one op)
        C'      = 0.5*u_f + u_i                  ( = 2c' )
        tau     = tanh(0.5 * C')                 (ACT free input scale)
        H'      = (th_o + 1) * tau               ( = 2h' )
  - gates live in two PSUM banks: [g,f,i] (needed first) and [o] (needed
    one hop later), so the gate tanh fires after only 12 of 16 matmuls.

Gate tile order everywhere: [g, f, i, o] (256 rows each).
"""

import numpy as np
import ml_dtypes

bf16 = ml_dtypes.bfloat16

B, T, DIN, H = 32, 512, 512, 256
NG = 4 * H          # 1024 gate rows per direction
NP = NG + H         # 1280 = gates + highway-half rows
BPC = 8             # samples per core
NCORES = 8
TOK = BPC * T       # tokens per core = 4096

# torch gate order i,f,g,o -> [g, f, i, o]
_PERM = np.concatenate([
    np.arange(512, 768),   # g
    np.arange(256, 512),   # f
    np.arange(0, 256),     # i
    np.arange(768, 1024),  # o
])
# rows that get the sigmoid-via-tanh 0.5 prescale (f, i, o; not g)
_HALF_ROWS = np.concatenate([
    np.zeros(256, dtype=bool),   # g
    np.ones(256, dtype=bool),    # f
    np.ones(256, dtype=bool),    # i
    np.ones(256, dtype=bool),    # o
])

_PROG_CACHE = {}


def _build_program(n_steps=T):
    import concourse.bacc as bacc
    import concourse.mybir as mybir
    import concourse.tile as tile

    fp32 = mybir.dt.float32
    b16 = mybir.dt.bfloat16
    AF = mybir.ActivationFunctionType
    OP = mybir.AluOpType

    nc = bacc.Bacc(None)

    xt_d = nc.dram_tensor("xt", [DIN, TOK], b16, kind="ExternalInput")
    wpt_d = nc.dram_tensor("wpt", [DIN, NP], b16, kind="ExternalInput")
    whht_d = nc.dram_tensor("whht", [H, NG], b16, kind="ExternalInput")
    bias_d = nc.dram_tensor("bias", [NP], fp32, kind="ExternalInput")
    ident_d = nc.dram_tensor("ident", [128, 128], fp32, kind="ExternalInput")
    out_d = nc.dram_tensor("out", [128, 2, T, BPC], fp32, kind="ExternalOutput")

    KT_A = DIN // 128      # 4 contraction tiles in phase A
    MT_A = NP // 128       # 10 output tiles in phase A (8 xg + 2 gpre)
    NCH_A = TOK // 512     # 8 token chunks of 512
    KT_B = H // 128        # 2 contraction tiles in recurrence
    CB = BPC               # one chain of all 8 samples

    with tile.TileContext(nc) as tc:
      with (
          tc.tile_pool(name="persist", bufs=1) as pp,
          tc.tile_pool(name="psum", bufs=2, space="PSUM") as psp,
      ):
        gpre = pp.tile([128, 2, T, BPC], fp32, tag="gpre")      # 32KB/p
        bias_sb = pp.tile([128, MT_A], fp32, tag="bias")
        nc.sync.dma_start(bias_sb[:], bias_d.rearrange("(m p) -> p m", p=128))

        whh_sb = pp.tile([128, KT_B, NG], b16, tag="whh")
        nc.sync.dma_start(whh_sb[:], whht_d.rearrange("(k p) m -> p k m", p=128))

        id_sb = pp.tile([128, 128], fp32, tag="ident")
        nc.sync.dma_start(id_sb[:], ident_d[:, :])

        yh = pp.tile([128, KT_B, n_steps + 1, CB], b16, tag="yh")
        # state tile: cols 0:2 = C (=2c, persistent), 2:8 = tanh of [g,f,i],
        # 8:10 = tanh of o
        th = pp.tile([128, 10, CB], fp32, tag="th")
        tau = pp.tile([128, 2, CB], fp32, tag="tau")
        nc.gpsimd.memset(yh[:, :, 0, :], 0.0)
        nc.gpsimd.memset(th[:, 0:2, :], 0.0)

        with tc.tile_pool(name="pxg", bufs=1) as pxg:
            # m-tile order [g,f,i,o] matches ps banks: 0:6 -> fig, 6:8 -> o
            xg = pxg.tile([128, 8, T, BPC], fp32, tag="xg")     # 128KB/p

            # ---------------- Phase A: projections ----------------
            with tc.tile_pool(name="phaseA", bufs=2) as pa:
                wp_sb = pa.tile([128, KT_A, NP], b16, tag="wp", bufs=1)
                nc.sync.dma_start(
                    wp_sb[:], wpt_d.rearrange("(k p) m -> p k m", p=128)
                )
                for n in range(NCH_A):
                    xt_sb = pa.tile([128, KT_A, 512], b16, tag="xt")
                    nc.sync.dma_start(
                        xt_sb[:],
                        xt_d.rearrange("(k p) n -> p k n", p=128)[
                            :, :, n * 512 : (n + 1) * 512
                        ],
                    )
                    for m in range(MT_A):
                        ps = psp.tile([128, 512], fp32, tag="psA", bufs=2)
                        for k in range(KT_A):
                            nc.tensor.matmul(
                                ps[:],
                                wp_sb[:, k, m * 128 : (m + 1) * 128],
                                xt_sb[:, k, :],
                                start=(k == 0),
                                stop=(k == KT_A - 1),
                            )
                        tchunk = ps[:].rearrange("p (t b) -> p t b", b=BPC)
                        t0 = n * (512 // BPC)
                        t1 = (n + 1) * (512 // BPC)
                        if m < 8:
                            dst = xg[:, m, t0:t1, :]
                        else:
                            dst = gpre[:, m - 8, t0:t1, :]
                        # alternate the PSUM->SBUF+bias epilogue between DVE
                        # and ACT so neither engine serializes all of phase A
                        if (n * MT_A + m) % 2 == 0:
                            nc.vector.tensor_scalar_add(
                                dst, tchunk, bias_sb[:, m : m + 1]
                            )
                        else:
                            nc.scalar.activation(
                                dst, tchunk, AF.Identity,
                                bias=bias_sb[:, m : m + 1],
                            )

            # ---------------- Phase B: recurrence ----------------
            with tc.tile_pool(name="phaseB", bufs=4) as pb:
                for t in range(n_steps):
                    tsl = slice(t, t + 1)
                    ps_fig = psp.tile([128, 6, CB], fp32, tag="psfig",
                                      bufs=3)
                    ps_o = psp.tile([128, 2, CB], fp32, tag="pso", bufs=3)
                    # xg injection: runs as soon as the PSUM buffer frees
                    # (no dependency on h) — off the critical path
                    nc.tensor.matmul(
                        ps_fig[:, :, :], id_sb[:, :], xg[:, 0:6, t, :],
                        start=True, stop=False,
                    )
                    nc.tensor.matmul(
                        ps_o[:, :, :], id_sb[:, :], xg[:, 6:8, t, :],
                        start=True, stop=False,
                    )
                    # h-dependent matmuls: g,f,i first (12), o last (4)
                    for m in range(8):
                        dst = (
                            ps_fig[:, m, :] if m < 6
                            else ps_o[:, m - 6, :]
                        )
                        for k in range(KT_B):
                            nc.tensor.matmul(
                                dst,
                                whh_sb[:, k, m * 128 : (m + 1) * 128],
                                yh[:, k, tsl, :],
                                start=False,
                                stop=(m in (5, 7) and k == KT_B - 1),
                                skip_group_check=True,
                            )
                        if m == 5:
                            nc.scalar.activation(
                                th[:, 2:8, :], ps_fig[:, :, :], AF.Tanh
                            )
                    nc.scalar.activation(
                        th[:, 8:10, :], ps_o[:, :, :], AF.Tanh
                    )
                    # u = (th_[f,i] + 1) * [C, th_g]   (paired lanes)
                    u = pb.tile([128, 2, 2, CB], fp32, tag="u")
                    nc.vector.scalar_tensor_tensor(
                        u[:],
                        th[:, 4:8, :].rearrange("p (a b) s -> p a b s", a=2),
                        1.0,
                        th[:, 0:4, :].rearrange("p (a b) s -> p a b s", a=2),
                        op0=OP.add,
                        op1=OP.mult,
                    )
                    # C' = 0.5 * u_f + u_i   (= 2c')
                    nc.vector.scalar_tensor_tensor(
                        th[:, 0:2, :], u[:, 0, :, :], 0.5,
                        u[:, 1, :, :], op0=OP.mult, op1=OP.add,
                    )
                    # tau = tanh(c') = tanh(0.5 * C')
                    nc.scalar.activation(
                        tau[:], th[:, 0:2, :], AF.Tanh, scale=0.5
                    )
                    # H' = (th_o + 1) * tau  (= 2h'), bf16 for the matmul
                    nc.vector.scalar_tensor_tensor(
                        yh[:, :, t + 1 : t + 2, :], th[:, 8:10, :], 1.0,
                        tau[:], op0=OP.add, op1=OP.mult,
                    )

        # ---------------- Phase C: highway gate ----------------
        with tc.tile_pool(name="phaseC", bufs=2) as pc:
            TC = 128
            for cch in range(T // TC):
                t0, t1 = cch * TC, (cch + 1) * TC
                gp = gpre[:, :, t0:t1, :]
                tg = pc.tile([128, 2, TC, BPC], fp32, tag="tg_c")
                nc.scalar.activation(tg[:], gp, AF.Sigmoid)
                # y = h = 0.5*H ;  yc = y - gpre
                yc = pc.tile([128, 2, TC, BPC], fp32, tag="y_c")
                nc.vector.scalar_tensor_tensor(
                    yc[:],
                    yh[:, :, t0 + 1 : t1 + 1, :], 0.5, gp,
                    op0=OP.mult, op1=OP.subtract,
                )
                fl = pc.tile([128, 2, TC, BPC], fp32, tag="fl_c")
                nc.vector.tensor_mul(fl[:], tg[:], yc[:])
                nc.vector.tensor_add(fl[:], fl[:], gp)
                nc.sync.dma_start(out_d[:, :, t0:t1, :], fl[:])

    nc.compile()
    return nc


def _reverse_padded_np(x, lens):
    t = np.arange(T)
    idx = np.where(t[None, :] < lens[:, None], lens[:, None] - 1 - t[None, :], t[None, :])
    return np.take_along_axis(x, idx[:, :, None], axis=1), idx


def kernel(x, Wih_f, Whh_f, bih_f, bhh_f, Wih_b, Whh_b, bih_b, bhh_b, Wg, bg,
           x_lengths, **_unused):
    from concourse.bass_utils import run_bass_kernel_spmd

    x = np.asarray(x, dtype=np.float32)
    lens = np.asarray(x_lengths).astype(np.int64)

    xr, idx = _reverse_padded_np(x, lens)

    rowscale = np.where(_HALF_ROWS, 0.5, 1.0)[:, None]  # [1024,1]

    def dir_weights(Wih, Whh, bih, bhh, wg_half, bg_half):
        Wihp = np.asarray(Wih)[_PERM] * rowscale            # rows: tanh trick
        Wp = np.concatenate([Wihp, wg_half], axis=0)        # [1280, 512]
        wpt = np.ascontiguousarray(Wp.T).astype(bf16)       # [512, 1280]
        # rows: perm + tanh prescale; cols: *0.5 because h is stored as 2h
        Whhp = np.asarray(Whh)[_PERM] * rowscale * 0.5
        whht = np.ascontiguousarray(Whhp.T).astype(bf16)    # [256,1024]
        bias = np.concatenate(
            [((np.asarray(bih) + np.asarray(bhh))[_PERM])
             * rowscale[:, 0], bg_half]
        ).astype(np.float32)
        return wpt, whht, bias

    Wg = np.asarray(Wg); bg = np.asarray(bg)
    fw = dir_weights(Wih_f, Whh_f, bih_f, bhh_f, Wg[0:H], bg[0:H])
    bw = dir_weights(Wih_b, Whh_b, bih_b, bhh_b, Wg[H:2*H], bg[H:2*H])

    ident = np.eye(128, dtype=np.float32)

    in_maps = []
    for c in range(NCORES):
        fwd = c < 4
        s0 = (c % 4) * BPC
        xsrc = x if fwd else xr
        xt = np.ascontiguousarray(
            xsrc[s0 : s0 + BPC].transpose(2, 1, 0).reshape(DIN, TOK)
        ).astype(bf16)
        wpt, whht, bias = fw if fwd else bw
        in_maps.append(
            {"xt": xt, "wpt": wpt, "whht": whht, "bias": bias, "ident": ident}
        )

    if "prog" not in _PROG_CACHE:
        _PROG_CACHE["prog"] = _build_program()
    nc = _PROG_CACHE["prog"]
    _PROG_CACHE["last_inmaps"] = in_maps

    res = run_bass_kernel_spmd(nc, in_maps, core_ids=list(range(NCORES)))

    full = np.zeros((B, T, 2 * H), dtype=np.float32)
    for c in range(NCORES):
        arr = np.asarray(res.results[c]["out"], dtype=np.float32)  # [128,2,T,BPC]
        half = arr.transpose(3, 2, 1, 0).reshape(BPC, T, H)
        s0 = (c % 4) * BPC
        if c < 4:
            full[s0 : s0 + BPC, :, 0:H] = half
        else:
            # un-reverse within valid lengths
            half = np.take_along_axis(half, idx[s0 : s0 + BPC][:, :, None], axis=1)
            full[s0 : s0 + BPC, :, H : 2 * H] = half

    mask = (np.arange(T)[None, :] < lens[:, None])[:, :, None]
    full *= mask
    return full
